# revision 1
# baseline (speedup 1.0000x reference)
"""Conv2d 3x3 (im2col GEMM) on 8 TRN2 NeuronCores.

Problem: x[16,64,112,112] (*) w[576,64] + b[64] -> out[16,64,112,112]
(3x3, stride 1, pad 1, NCHW, im2col patch order (c, kh, kw)).

Strategy
--------
Data-parallel over batch: 2 images per core, 8 cores, no collectives.

Per image, an implicit-GEMM formulation that needs only 3 full-width
fp32r matmuls per 448 outputs (vs 9 for naive per-tap GEMM):

  * x is staged in SBUF as z[128, F]: partitions 0:64 hold the image
    flattened row-major with a 113-element zero pad at each end
    ("zt"), partitions 64:128 hold the same data shifted left by one
    element ("zb", built by SBUF->SBUF DMA). Loads and shift-copies are
    issued in 4 row segments so compute overlaps the load.
  * For each kh in {0,1,2} one matmul with a block lhsT
        [[w(kh,1), w(kh,0)],
         [w(kh,2),    0   ]]
    accumulates into one 450-col psum chunk (4 output rows):
      psum[0:64,  j] += taps (kw=1 via zt) + (kw=2 via zb)  of out[s+j]
      psum[64:128,j] += tap  (kw=0 via zt)                  of out[s+j+1]
    Chunks are processed two-at-a-time in a [128, 1024] psum tile
    (one bank per chunk) so ACT/DVE post-ops run at 896 elements/op.
  * ACT adds bias to psum[64:128], DVE folds the two halves ->
    complete conv outputs.
  * Row-major flattening wraps at image-row boundaries, so the kw=0/2
    taps of the first/last column of each row pick up a neighbor-row
    value; tiny strided matmuls per half-image recompute exactly those
    terms (reading the same SBUF words) and DVE subtracts them.

Inputs are pre-rounded on the host to the fp32r grid (11-bit mantissa)
so every device-side producer of matmul data is a pure bit-copy, which
walrus' fp32r verifier accepts; psum accumulation stays full fp32.
"""

import numpy as np

import concourse.bacc as bacc
import concourse.mybir as mybir
import concourse.tile as tile
from concourse import bass_utils

# problem geometry (hardcoded per contract)
B, CIN, H, W = 16, 64, 112, 112
COUT = 64
NCORES = 8
IMGS = B // NCORES  # images per core

HW = H * W                     # 12544
ZOFF = W + 1                   # lead zero pad: 1 + one full pad row
F = ZOFF + HW + ZOFF           # z free size per image (12770)
ROWS_PER_CHUNK = 4
CHUNK = ROWS_PER_CHUNK * W     # 448 outputs per psum chunk
MMW = CHUNK + 2                # 450: matmul moving width (even, fp32r rule)
NCHUNK = H // ROWS_PER_CHUNK   # 28
NPAIR = NCHUNK // 2            # 14 chunk pairs per image
PAIRS_PER_HALF = NPAIR // 2    # 7
HALF_ROWS = H // 2             # 56
HALF = HALF_ROWS * W           # 6272 outputs per store half
# z load segments (z-flat cut points); each covers 28 rows + 2-row halo
SEG_END = [ZOFF + min(28 * (g + 1) + 2, H) * W for g in range(4)]
# z alloc pad so the (sliced-then-strided) correction rhs views stay in
# bounds; the strided APs themselves never read past F.
F_ALLOC = F + 111

f32 = mybir.dt.float32
f32r = mybir.dt.float32r
u32 = mybir.dt.uint32

_cache = {}


def _round_f32r(a: np.ndarray) -> np.ndarray:
    """Round fp32 to the fp32r grid (11 mantissa bits, RNE)."""
    u = np.ascontiguousarray(a, dtype=np.float32).view(np.uint32).copy()
    lsb = (u >> 12) & 1
    u += 0x7FF + lsb
    u &= np.uint32(0xFFFFF000)
    return u.view(np.float32)


def _prep_weights(weight):
    """Host-side: block lhsT matrices + kw=2 slices, fp32r-rounded."""
    w_r = _round_f32r(weight).reshape(CIN, 3, 3, COUT)  # [c, kh, kw, m]
    lt = np.zeros((3, 128, 128), np.float32)
    w2 = np.zeros((3, 64, 64), np.float32)
    for kh in range(3):
        lt[kh, 0:64, 0:64] = w_r[:, kh, 1, :]
        lt[kh, 0:64, 64:128] = w_r[:, kh, 0, :]
        lt[kh, 64:128, 0:64] = w_r[:, kh, 2, :]
        w2[kh] = w_r[:, kh, 2, :]
    return lt, w2


def _build(repeat=None):
    nc = bacc.Bacc("TRN2", target_bir_lowering=False, debug=False,
                   num_devices=NCORES)

    x_d = nc.dram_tensor("x", (IMGS, CIN, H, W), f32r, kind="ExternalInput")
    lt_d = nc.dram_tensor("lhsT", (3, 128, 128), f32r, kind="ExternalInput")
    w2_d = nc.dram_tensor("w2s", (3, 64, 64), f32r, kind="ExternalInput")
    b_d = nc.dram_tensor("bias", (COUT,), f32, kind="ExternalInput")
    o_d = nc.dram_tensor("out", (IMGS, COUT, H, W), f32, kind="ExternalOutput")

    xv = x_d.ap().rearrange("b c h w -> b c (h w)")
    ov = o_d.ap().rearrange("b c h w -> b c (h w)")

    with tile.TileContext(nc) as tc:
        with (
            tc.tile_pool(name="wpool", bufs=1) as wpool,
            tc.tile_pool(name="zpool", bufs=2) as zpool,
            tc.tile_pool(name="opool", bufs=3) as opool,
            tc.tile_pool(name="tpool", bufs=int(__import__("os").environ.get("TB_BUFS", "4"))) as tpool,
            tc.tile_pool(name="ppool", bufs=3, space="PSUM") as ppool,
            tc.tile_pool(name="cpool", bufs=1, space="PSUM") as cpool,
        ):
            # --- weights / bias staging (once) ---
            bias = wpool.tile([COUT, 1], f32)
            nc.sync.dma_start(
                bias[:, :], b_d.ap().rearrange("(c one) -> c one", one=1))

            lhs = []
            w2 = []
            for kh in range(3):
                lt = wpool.tile([128, 128], f32r, name=f"lhsT{kh}",
                                tag=f"lhsT{kh}")
                nc.sync.dma_start(lt[:, :], lt_d.ap()[kh])
                lhs.append(lt)
                w2t = wpool.tile([64, 64], f32r, name=f"w2_{kh}",
                                 tag=f"w2_{kh}")
                nc.sync.dma_start(w2t[:, :], w2_d.ap()[kh])
                w2.append(w2t)

            import contextlib
            loop_cm = (
                tc.For_i(0, repeat, 1)
                if repeat is not None else contextlib.nullcontext()
            )
            with loop_cm:
              import os as _os
              prep_ahead = _os.environ.get("PREP_AHEAD", "0") == "1"
              zs = {}

              def prep(img):
                  z = zpool.tile([128, F_ALLOC], f32r, name="z", tag="z")
                  zs[img] = z
                  # zero pads
                  nc.vector.memset(z[:, 0:ZOFF].bitcast(u32), 0)
                  nc.vector.memset(z[0:64, ZOFF + HW: F].bitcast(u32), 0)
                  nc.vector.memset(z[64:128, ZOFF + HW - 1: F].bitcast(u32), 0)
                  # segmented zt load + zb shift copy (zb[i] = zt[i+1]);
                  # copies alternate SP-HWDGE / SWDGE rings to balance the
                  # two DMA issue queues (loads on SP, stores on SWDGE)
                  import os as _os3
                  if _os3.environ.get("SEG_FINE", "0") == "1":
                      seg_end = [ZOFF + min(r + 2, H) * W
                                 for r in (10, 38, 66, 94, H)]
                  else:
                      seg_end = SEG_END
                  segi = 0
                  prev = ZOFF
                  for end in seg_end:
                      nc.sync.dma_start(z[0:64, prev:end],
                                        xv[img][:, prev - ZOFF: end - ZOFF])
                      a0 = prev - 1
                      ceng = nc.sync if segi % 2 == 0 else nc.gpsimd
                      ceng.dma_start(z[64:128, a0: end - 1],
                                     z[0:64, a0 + 1: end])
                      segi += 1
                      prev = end

              if prep_ahead:
                  for img in range(IMGS):
                      prep(img)
              for img in range(IMGS):
                  if not prep_ahead:
                      prep(img)
                  z = zs[img]

                  for half in range(2):
                      og = opool.tile([COUT, HALF], f32, name="og", tag="og")
                      for pp in range(PAIRS_PER_HALF):
                          p = half * PAIRS_PER_HALF + pp
                          ps = ppool.tile([128, 1024], f32, name="ps", tag="ps")
                          for k in range(2):
                              y0 = (2 * p + k) * ROWS_PER_CHUNK
                              for kh in range(3):
                                  a = (y0 + kh) * W
                                  nc.tensor.matmul(
                                      ps[:, 512 * k: 512 * k + MMW],
                                      lhs[kh][:, :],
                                      z[:, a: a + MMW],
                                      start=(kh == 0),
                                      stop=(kh == 2),
                                  )
                          psv = ps[:, :].rearrange("q (a b) -> q a b", b=512)
                          tb = tpool.tile([COUT, 2 * CHUNK], f32, name="tb",
                                          tag="tb")
                          tbv = tb[:, :].rearrange("q (a b) -> q a b", b=CHUNK)
                          nc.scalar.add(tbv, psv[64:128, :, 0:CHUNK],
                                        bias[:, :])
                          ogv = og[:, pp * 2 * CHUNK: (pp + 1) * 2 * CHUNK
                                   ].rearrange("q (a b) -> q a b", b=CHUNK)
                          psa = ps[:, :].rearrange(
                              "q (a b) -> q a b", b=512)[0:64, :, 1: CHUNK + 1]
                          nc.vector.tensor_add(ogv, psa, tbv)

                      # --- edge corrections for rows y = 56*half .. +55 ---
                      pc1 = cpool.tile([64, HALF_ROWS], f32, name="pc1",
                                       tag="pc1")
                      pc2 = cpool.tile([64, HALF_ROWS], f32, name="pc2",
                                       tag="pc2")
                      for kh in range(3):
                          a = (HALF_ROWS * half + kh) * W
                          rhs = z[0:64, a: a + HALF_ROWS * W].rearrange(
                              "q (r w) -> q r w", w=W)[:, :, 0]
                          nc.tensor.matmul(
                              pc1[:, :], lhs[kh][0:64, 64:128], rhs,
                              start=(kh == 0), stop=(kh == 2))
                      for kh in range(3):
                          a = ZOFF + (HALF_ROWS * half + kh) * W
                          rhs = z[0:64, a: a + HALF_ROWS * W].rearrange(
                              "q (r w) -> q r w", w=W)[:, :, 0]
                          nc.tensor.matmul(
                              pc2[:, :], w2[kh][:, :], rhs,
                              start=(kh == 0), stop=(kh == 2))
                      ogr = og[:, :].rearrange("q (r w) -> q r w", w=W)
                      col0 = ogr[:, :, 0]
                      col_last = ogr[:, :, W - 1]
                      nc.vector.tensor_sub(col0, col0, pc1[:, :])
                      nc.vector.tensor_sub(col_last, col_last, pc2[:, :])

                      # store via SWDGE so the SP-HWDGE ring stays free for
                      # prefetch loads (stores on SP/ACT rings measured much
                      # slower / crashed)
                      nc.gpsimd.dma_start(
                          ov[img, :, half * HALF: (half + 1) * HALF], og[:, :])

    nc.compile()
    return nc


def kernel(x: np.ndarray, weight: np.ndarray, bias: np.ndarray,
           **_ignored) -> np.ndarray:
    if "nc" not in _cache:
        _cache["nc"] = _build()
    nc = _cache["nc"]

    x_r = _round_f32r(x).reshape(B, CIN, H, W)
    lt_np, w2_np = _prep_weights(weight)
    b_np = np.ascontiguousarray(bias, dtype=np.float32)

    in_maps = [
        {
            "x": np.ascontiguousarray(x_r[i * IMGS: (i + 1) * IMGS]),
            "lhsT": lt_np,
            "w2s": w2_np,
            "bias": b_np,
        }
        for i in range(NCORES)
    ]
    res = bass_utils.run_bass_kernel_spmd(
        nc, in_maps, core_ids=list(range(NCORES)))
    out = np.concatenate([r["out"] for r in res.results], axis=0)
    return out.reshape(B, COUT, H, W)



# revision 7
# speedup vs baseline: 51.6207x; 51.6207x over previous
"""Conv2d 3x3 (im2col GEMM) on 8 TRN2 NeuronCores.

Problem: x[16,64,112,112] (*) w[576,64] + b[64] -> out[16,64,112,112]
(3x3, stride 1, pad 1, NCHW, im2col patch order (c, kh, kw)).

Strategy
--------
Data-parallel over batch: 2 images per core, 8 cores, no collectives.

Per image, an implicit-GEMM formulation that needs only 3 full-width
matmuls per 448 outputs (vs 9 for naive per-tap GEMM):

  * x is staged in SBUF as z[128, F] in bf16: partitions 0:64 hold the
    image flattened row-major with a 113-element zero pad at each end
    ("zt"), partitions 64:128 hold the same data shifted left by one
    element ("zb", built by SBUF->SBUF DMA). Loads and shift-copies are
    issued in 4 row segments so compute overlaps the load.
  * For each kh in {0,1,2} one matmul with a block lhsT
        [[w(kh,1), w(kh,0)],
         [w(kh,2),    0   ]]
    accumulates into one 450-col psum chunk (4 output rows):
      psum[0:64,  j] += taps (kw=1 via zt) + (kw=2 via zb)  of out[s+j]
      psum[64:128,j] += tap  (kw=0 via zt)                  of out[s+j+1]
    Chunks are processed two-at-a-time in a [128, 1024] psum tile
    (one bank per chunk) so ACT/DVE post-ops run at 896 elements/op.
  * ACT adds bias to psum[64:128], DVE folds the two halves ->
    complete conv outputs (written as bf16).
  * Row-major flattening wraps at image-row boundaries, so the kw=0/2
    taps of the first/last column of each row pick up a neighbor-row
    value; tiny strided matmuls per half-image recompute exactly those
    terms (reading the same SBUF words) and DVE subtracts them.

bf16 everywhere on the wire (x, weights, output) with fp32 psum
accumulation: halves every DMA transfer vs fp32 - the DMA pool was the
binding resource of the fp32 version (54us of 74us span). Quantization
error ~2e-3 relative, well under the 2e-2 gate. All weights ship as ONE
DMA (packed [128, 576] tensor) so the first x segment load is not
queued behind 7 small weight transfers.
"""

import numpy as np
import ml_dtypes

import concourse.bacc as bacc
import concourse.mybir as mybir
import concourse.tile as tile
from concourse import bass_utils

# problem geometry (hardcoded per contract)
B, CIN, H, W = 16, 64, 112, 112
COUT = 64
NCORES = 8
IMGS = B // NCORES  # images per core

HW = H * W                     # 12544
ZOFF = W + 1                   # lead zero pad: 1 + one full pad row
F = ZOFF + HW + ZOFF           # z free size per image (12770)
ROWS_PER_CHUNK = 4
CHUNK = ROWS_PER_CHUNK * W     # 448 outputs per psum chunk
MMW = CHUNK + 2                # 450: matmul moving width
NCHUNK = H // ROWS_PER_CHUNK   # 28
NPAIR = NCHUNK // 2            # 14 chunk pairs per image
PAIRS_PER_HALF = NPAIR // 2    # 7
HALF_ROWS = H // 2             # 56
HALF = HALF_ROWS * W           # 6272 outputs per store half
# z load segments (z-flat cut points); each covers 28 rows + 2-row halo
SEG_END = [ZOFF + min(28 * (g + 1) + 2, H) * W for g in range(4)]
# z alloc pad so the (sliced-then-strided) correction rhs views stay in
# bounds; the strided APs themselves never read past F.
F_ALLOC = F + 111

f32 = mybir.dt.float32
bf16 = mybir.dt.bfloat16
u16 = mybir.dt.uint16

_cache = {}

bfloat16 = ml_dtypes.bfloat16


def _prep_weights(weight):
    """Host-side: pack block lhsT matrices + kw=2 slices into one
    [128, 3*128 + 3*64] bf16 tensor (single DMA)."""
    w_r = np.asarray(weight, np.float32).astype(bfloat16)
    w_r = w_r.reshape(CIN, 3, 3, COUT)  # [c, kh, kw, m]
    wt = np.zeros((128, 3 * 128 + 3 * 64), bfloat16)
    for kh in range(3):
        blk = wt[:, kh * 128: (kh + 1) * 128]
        blk[0:64, 0:64] = w_r[:, kh, 1, :]
        blk[0:64, 64:128] = w_r[:, kh, 0, :]
        blk[64:128, 0:64] = w_r[:, kh, 2, :]
        wt[0:64, 384 + kh * 64: 384 + (kh + 1) * 64] = w_r[:, kh, 2, :]
    return wt


def _build(repeat=None):
    nc = bacc.Bacc("TRN2", target_bir_lowering=False, debug=False,
                   num_devices=NCORES)

    x_d = nc.dram_tensor("x", (IMGS, CIN, H, W), bf16, kind="ExternalInput")
    wt_d = nc.dram_tensor("wts", (128, 576), bf16, kind="ExternalInput")
    b_d = nc.dram_tensor("bias", (COUT,), f32, kind="ExternalInput")
    o_d = nc.dram_tensor("out", (IMGS, COUT, H, W), bf16,
                         kind="ExternalOutput")

    xv = x_d.ap().rearrange("b c h w -> b c (h w)")
    ov = o_d.ap().rearrange("b c h w -> b c (h w)")

    with tile.TileContext(nc) as tc:
        with (
            tc.tile_pool(name="wpool", bufs=1) as wpool,
            tc.tile_pool(name="zpool", bufs=1) as zpool,
            tc.tile_pool(name="opool", bufs=3) as opool,
            tc.tile_pool(name="tpool", bufs=4) as tpool,
            tc.tile_pool(name="ppool", bufs=3, space="PSUM") as ppool,
            tc.tile_pool(name="cpool", bufs=2, space="PSUM") as cpool,
        ):
            # --- weights / bias staging (one big + one tiny DMA, split
            # across the two DGE rings so neither blocks x loads long) ---
            wt = wpool.tile([128, 576], bf16, name="wt", tag="wt")
            nc.gpsimd.dma_start(wt[:, :], wt_d.ap())
            bias = wpool.tile([COUT, 1], f32)
            nc.gpsimd.dma_start(
                bias[:, :], b_d.ap().rearrange("(c one) -> c one", one=1))

            lhs = [wt[:, kh * 128: (kh + 1) * 128] for kh in range(3)]
            w2 = [wt[0:64, 384 + kh * 64: 384 + (kh + 1) * 64]
                  for kh in range(3)]

            # Persistent z tiles (one per image, no rotation): the zero
            # pads are invariant, so memset them ONCE here instead of per
            # For_i iteration - per-iter pad memsets were head-of-line
            # blocking the next iteration's staging DMAs.
            zs = [zpool.tile([128, F_ALLOC], bf16, name=f"z{i}", tag=f"z{i}")
                  for i in range(IMGS)]
            for z in zs:
                nc.vector.memset(z[:, 0:ZOFF].bitcast(u16), 0)
                nc.vector.memset(z[0:64, ZOFF + HW: F].bitcast(u16), 0)
                nc.vector.memset(z[64:128, ZOFF + HW - 1: F].bitcast(u16), 0)

            import contextlib
            loop_cm = (
                tc.For_i(0, repeat, 1)
                if repeat is not None else contextlib.nullcontext()
            )
            with loop_cm:
              def prep(img):
                  # segmented zt load + zb shift copy (zb[i] = zt[i+1]),
                  # all on the SP-HWDGE ring: seg0's copy right after its
                  # load (starts compute ASAP), remaining copies after all
                  # loads so one copy's sem wait never head-of-line blocks
                  # a load. Stores live on the SWDGE ring so a pending
                  # store never blocks staging.
                  z = zs[img]
                  prev = ZOFF
                  copies = []
                  for gi, end in enumerate(SEG_END):
                      nc.sync.dma_start(z[0:64, prev:end],
                                        xv[img][:, prev - ZOFF: end - ZOFF])
                      a0 = prev - 1
                      if gi == 0:
                          nc.sync.dma_start(z[64:128, a0: end - 1],
                                            z[0:64, a0 + 1: end])
                      else:
                          copies.append((a0, end))
                      prev = end
                  for a0, end in copies:
                      nc.sync.dma_start(z[64:128, a0: end - 1],
                                        z[0:64, a0 + 1: end])

              for img in range(IMGS):
                  prep(img)
                  z = zs[img]

                  for half in range(2):
                      og = opool.tile([COUT, HALF], bf16, name="og", tag="og")
                      for pp in range(PAIRS_PER_HALF):
                          p = half * PAIRS_PER_HALF + pp
                          ps = ppool.tile([128, 1024], f32, name="ps",
                                          tag="ps")
                          for k in range(2):
                              y0 = (2 * p + k) * ROWS_PER_CHUNK
                              for kh in range(3):
                                  a = (y0 + kh) * W
                                  nc.tensor.matmul(
                                      ps[:, 512 * k: 512 * k + MMW],
                                      lhs[kh],
                                      z[:, a: a + MMW],
                                      start=(kh == 0),
                                      stop=(kh == 2),
                                  )
                          psv = ps[:, :].rearrange("q (a b) -> q a b", b=512)
                          tb = tpool.tile([COUT, 2 * CHUNK], f32, name="tb",
                                          tag="tb")
                          tbv = tb[:, :].rearrange("q (a b) -> q a b",
                                                   b=CHUNK)
                          nc.scalar.add(tbv, psv[64:128, :, 0:CHUNK],
                                        bias[:, :])
                          ogv = og[:, pp * 2 * CHUNK: (pp + 1) * 2 * CHUNK
                                   ].rearrange("q (a b) -> q a b", b=CHUNK)
                          psa = ps[:, :].rearrange(
                              "q (a b) -> q a b", b=512)[0:64, :, 1: CHUNK + 1]
                          nc.vector.tensor_add(ogv, psa, tbv)

                      # --- edge corrections for rows y = 56*half .. +55 ---
                      pc = cpool.tile([64, 2 * HALF_ROWS], f32, name="pc",
                                      tag="pc")
                      pc1 = pc[:, 0:HALF_ROWS]
                      pc2 = pc[:, HALF_ROWS: 2 * HALF_ROWS]
                      for kh in range(3):
                          a = (HALF_ROWS * half + kh) * W
                          rhs = z[0:64, a: a + HALF_ROWS * W].rearrange(
                              "q (r w) -> q r w", w=W)[:, :, 0]
                          nc.tensor.matmul(
                              pc1, lhs[kh][0:64, 64:128], rhs,
                              start=(kh == 0), stop=(kh == 2))
                      for kh in range(3):
                          a = ZOFF + (HALF_ROWS * half + kh) * W
                          rhs = z[0:64, a: a + HALF_ROWS * W].rearrange(
                              "q (r w) -> q r w", w=W)[:, :, 0]
                          nc.tensor.matmul(
                              pc2, w2[kh], rhs,
                              start=(kh == 0), stop=(kh == 2))
                      ogr = og[:, :].rearrange("q (r w) -> q r w", w=W)
                      col0 = ogr[:, :, 0]
                      col_last = ogr[:, :, W - 1]
                      nc.vector.tensor_sub(col0, col0, pc1)
                      nc.vector.tensor_sub(col_last, col_last, pc2)

                      # store via SWDGE so the SP-HWDGE ring stays free for
                      # prefetch loads
                      nc.gpsimd.dma_start(
                          ov[img, :, half * HALF: (half + 1) * HALF],
                          og[:, :])

    nc.compile()
    return nc


def prep_in_maps(x, weight, bias):
    x_bf = np.ascontiguousarray(x, np.float32).astype(bfloat16)
    x_bf = x_bf.reshape(B, CIN, H, W)
    wt_np = _prep_weights(weight)
    b_np = np.ascontiguousarray(bias, dtype=np.float32)
    return [
        {
            "x": np.ascontiguousarray(x_bf[i * IMGS: (i + 1) * IMGS]),
            "wts": wt_np,
            "bias": b_np,
        }
        for i in range(NCORES)
    ]


def kernel(x: np.ndarray, weight: np.ndarray, bias: np.ndarray,
           **_ignored) -> np.ndarray:
    if "nc" not in _cache:
        _cache["nc"] = _build()
    nc = _cache["nc"]

    in_maps = prep_in_maps(x, weight, bias)
    res = bass_utils.run_bass_kernel_spmd(
        nc, in_maps, core_ids=list(range(NCORES)))
    out = np.concatenate(
        [np.asarray(r["out"]).astype(np.float32) for r in res.results],
        axis=0)
    return out.reshape(B, COUT, H, W)
